# revision 19
# baseline (speedup 1.0000x reference)
"""Trainium2 Bass kernel for nn_AttentionModule (sparse axial-pooled attention).

Strategy: data-parallel over batch B=16 across 8 NeuronCores (2 images per
core), one SPMD program, no collectives.

Per image (H*W = 4096 pixels, C = 512):
  1. x arrives bf16; PE "transposes" each [128pix,128c] chunk via a plain
     matmul against an extended identity [I128 | 1]: out col 128 carries the
     per-chunk pixel-sum, so xsum rides the transpose for free.
  2. q = (xsum/4096) @ Wq + bq; wqk = fold of Wk with q (host passes Wk^T).
  3. Scores in natural pixel-major layout: s[pix, n] via stationary-xT
     matmuls with tiny free-8 outputs, exp'd in groups of 4 tiles on Act.
  4. V = xT @ Wv (f32r weights, full PE rate); wt = E * V elementwise
     (DVE/Pool); masked-sum matmuls give softmax numerators/denominators.
  5. A_h/A_v normalize, transpose (plain-mm vs bf16 identity), combine via
     broadcast products (DVE/Pool) + stacked-identity pair-sum matmul.
  6. out = A @ [Wo; bo]; result DMA'd straight from PSUM to DRAM.
"""

import sys

sys.path.insert(0, "/opt/trn_rl_repo")

import numpy as np
import ml_dtypes

import concourse.bass as bass
import concourse.tile as tile
from concourse import bacc, mybir
from concourse import bass_utils

F32 = mybir.dt.float32
F32R = mybir.dt.float32r
BF16 = mybir.dt.bfloat16
BF = ml_dtypes.bfloat16

B, H, W, C = 16, 64, 64, 512
NHEAD, DK, DV, DO = 8, 64, 64, 512
NCORES = 8
BPC = B // NCORES          # images per core
NPIX = H * W               # 4096
NTILES = NPIX // 128       # 32 pixel tiles per image
NBLK = NPIX // 512         # 8 pixel blocks per image


def _build_kernel():
    nc = bacc.Bacc("TRN2", target_bir_lowering=False, debug=False)

    dram = {}
    def din(name, shape, dt=F32):
        dram[name] = nc.dram_tensor(name, list(shape), dt, kind="ExternalInput").ap()
        return dram[name]

    x_d = din("x", (BPC, NPIX, C), BF16)
    wq_d = din("Wq", (C, NHEAD * DK))
    wkT_d = din("WkT", (NHEAD * DK, C))
    wv_d = din("Wv", (C, NHEAD * DV))
    woe_d = din("Wo_ext", (DV + 1, DO))      # [Wo; bo]
    bq_d = din("bq", (NHEAD * DK,))
    bv_d = din("bv", (NHEAD * DV,))
    idx_d = din("idext", (128, 129), BF16)   # [I128 | ones] for transpose+sum
    ii_d = din("ii64", (128, 64), BF16)      # two stacked 64-identities
    msk_d = din("masks", (NTILES, 128, 128), BF16)

    out_d = nc.dram_tensor("out", [BPC, NPIX, DO], BF16, kind="ExternalOutput").ap()

    with tile.TileContext(nc) as tc:
        _body(tc, x_d, wq_d, wkT_d, wv_d, woe_d, bq_d, bv_d,
              idx_d, ii_d, msk_d, out_d)

    nc.compile()
    return nc


def _body(tc, x_d, wq_d, wkT_d, wv_d, woe_d, bq_d, bv_d,
          idx_d, ii_d, msk_d, out_d):
    nc = tc.nc
    from contextlib import ExitStack
    ctx = ExitStack()

    const = ctx.enter_context(tc.tile_pool(name="const", bufs=1))
    xtp = ctx.enter_context(tc.tile_pool(name="xtp", bufs=1))
    xload = ctx.enter_context(tc.tile_pool(name="xload", bufs=8))
    epool = ctx.enter_context(tc.tile_pool(name="epool", bufs=10))
    wpool = ctx.enter_context(tc.tile_pool(name="wpool", bufs=3))
    small = ctx.enter_context(tc.tile_pool(name="small", bufs=2))
    att = ctx.enter_context(tc.tile_pool(name="att", bufs=2))
    ppool = ctx.enter_context(tc.tile_pool(name="ppool", bufs=6))
    atpool = ctx.enter_context(tc.tile_pool(name="atpool", bufs=1))

    # PSUM: 8 banks.  tp(2) + sc(1) + v(1) + nhv(1) + big(3) = 8
    ps_tp = ctx.enter_context(tc.tile_pool(name="ps_tp", bufs=2, space="PSUM"))
    ps_sc = ctx.enter_context(tc.tile_pool(name="ps_sc", bufs=1, space="PSUM"))
    ps_v = ctx.enter_context(tc.tile_pool(name="ps_v", bufs=2, space="PSUM"))
    ps_nhv = ctx.enter_context(tc.tile_pool(name="ps_nhv", bufs=1, space="PSUM"))
    ps_big = ctx.enter_context(tc.tile_pool(name="ps_big", bufs=2, space="PSUM"))

    def issue_x_dma_early(g):
        xt = xload.tile([128, 4, 512], BF16, tag="xt")
        nc.sync.dma_start(
            xt[:], x_d[0, g * 512:(g + 1) * 512, :]
            .rearrange("(t p) c -> p t c", p=128))
        return xt

    # ---- constants into SBUF (once per core); x DMAs are emitted first
    # in phase_load, so only idext goes ahead of them ----
    idx_sb = const.tile([128, 129], BF16, tag="idx")
    nc.sync.dma_start(idx_sb[:], idx_d)
    ii_sb = const.tile([128, 64], BF16, tag="ii")
    nc.sync.dma_start(ii_sb[:], ii_d)

    _prefetch0 = [issue_x_dma_early(g) for g in range(NBLK)]

    def load_r(shape, tag, src):
        stage = xload.tile(list(shape), F32, tag="wstage", bufs=1)
        nc.sync.dma_start(stage[:], src)
        t = const.tile(list(shape), F32R, tag=tag)
        nc.vector.tensor_copy(t[:], stage[:])
        return t

    wvstage = xload.tile([128, 4, 512], F32, name="wvstage", tag="wstage", bufs=1)
    _dummy = None
    nc.sync.dma_start(wvstage[:], wv_d.rearrange("(j p) c -> p j c", p=128))
    wv4 = const.tile([128, 4, 512], BF16, name="wv4", tag="wv4")
    nc.vector.tensor_copy(wv4[:], wvstage[:])
    wv_sb = [wv4[:, j, :] for j in range(4)]
    woe_sb = load_r([DV + 1, DO], "woe", woe_d)

    msk_sb = const.tile([128, NTILES, 128], BF16, name="msk", tag="msk")
    nc.sync.dma_start(msk_sb[:], msk_d.transpose([1, 0, 2]))

    wq4 = const.tile([128, 4, 512], F32, name="wq4", tag="wq4")
    nc.sync.dma_start(wq4[:], wq_d.rearrange("(j p) c -> p j c", p=128))
    wkT4 = const.tile([128, 4, 512], F32, name="wkT4", tag="wkT4")
    nc.sync.dma_start(wkT4[:], wkT_d.rearrange("(j p) c -> p j c", p=128))
    wq_sb = [wq4[:, j, :] for j in range(4)]
    wkT = [wkT4[:, j, :] for j in range(4)]
    bq4 = const.tile([128, 4], F32, name="bq4", tag="bq4")
    nc.sync.dma_start(bq4[:], bq_d.rearrange("(j p) -> p j", p=128))
    bq_sb = [bq4[:, j:j + 1] for j in range(4)]

    bv0 = const.tile([1, 512], F32, tag="bv0")
    nc.sync.dma_start(bv0[:], bv_d.unsqueeze(0))
    bv_rep = const.tile([64, 512], F32, tag="bvrep")
    nc.gpsimd.partition_broadcast(bv_rep[:], bv0[:])

    id64r = const.tile([64, 64], F32R, tag="id64r")
    nc.vector.tensor_copy(id64r[:], idx_sb[0:64, 0:64])
    iir = const.tile([128, 64], F32R, tag="iir")
    nc.vector.tensor_copy(iir[:], ii_sb[:])
    z32 = const.tile([128, 8], F32, tag="z32")
    nc.vector.memset(z32[:], 0.0)
    ones32 = const.tile([1, 512], F32, tag="ones32")
    nc.vector.memset(ones32[:], 1.0)

    # persistent per-image state
    xT = [xtp.tile([128, 4, NPIX], BF16, name=f"xT{b}", tag=f"xT{b}")
          for b in range(BPC)]

    # at-tiles: ones row written once, [0:64] rewritten per block
    at_tiles = [atpool.tile([DV + 1, 512], F32R, name=f"at{i}", tag=f"at{i}")
                for i in range(2)]
    for i in range(2):
        nc.scalar.activation(at_tiles[i][64:65, :], ones32[:],
                             mybir.ActivationFunctionType.Copy)

    # ---------------- per-image phases ----------------
    def issue_x_dma(b, g):
        xt = xload.tile([128, 4, 512], BF16, tag="xt")
        nc.sync.dma_start(
            xt[:], x_d[b, g * 512:(g + 1) * 512, :]
            .rearrange("(t p) c -> p t c", p=128))
        return xt

    def load_macro(b, g, xs32, xt=None):
        """One 4-pixel-tile macro: transposes + evac + pixel-sums.
        Rotate transpose psum over pools idle in this phase."""
        if xt is None:
            xt = issue_x_dma(b, g)
        if b == 0:
            rot = [(ps_tp, "tp"), (ps_tp, "tp"), (ps_v, "v"), (ps_v, "v"),
                   (ps_sc, "scd"), (ps_nhv, "nhv")]
        else:
            rot = [(ps_tp, "tp"), (ps_tp, "tp"),
                   (ps_sc, "scd"), (ps_nhv, "nhv")]
        ri = 0
        for t in range(4 * g, 4 * g + 4):
            for half in range(2):
                pool, ptag = rot[(8 * (g % len(rot)) + ri) % len(rot)]
                ri += 1
                ps = pool.tile([128, 2, 129], F32, tag=ptag)
                for j in range(2):
                    jj = half * 2 + j
                    nc.tensor.matmul(ps[:, j, :],
                                     xt[:, t % 4, jj * 128:(jj + 1) * 128],
                                     idx_sb[:], start=True, stop=True)
                dst = xT[b][:, half * 2:half * 2 + 2, t * 128:(t + 1) * 128]
                srcv = ps[:, :, 0:128]
                mod = 4 if b == 0 else 2
                if (2 * t + half) % mod != mod - 1:
                    nc.scalar.activation(dst, srcv,
                                         mybir.ActivationFunctionType.Copy)
                else:
                    nc.vector.tensor_copy(dst, srcv)
                sums = ps[:, :, 128:129]
                v = xs32[:, half * 2:half * 2 + 2].unsqueeze(2)
                if t == 0:
                    nc.vector.tensor_copy(v, sums)
                else:
                    nc.vector.tensor_add(v, v, sums)

    def phase_load(b, prefetched=None):
        xs32 = small.tile([128, 4], F32, name=f"xs{b}", tag=f"xs{b}")
        for g in range(NBLK):
            load_macro(b, g, xs32,
                       prefetched[g] if prefetched is not None else None)
        return xs32

    def phase_q(b, xs32):
        """q = (xsum/4096) @ Wq + bq; wqk[c, n] fold (bf16)."""
        qt_sb = []
        for jo in range(4):
            qp = ps_sc.tile([128, 1], F32, tag="scd")
            for j in range(4):
                nc.tensor.matmul(qp[:],
                                 wq_sb[j][:, jo * 128:(jo + 1) * 128],
                                 xs32[:, j:j + 1],
                                 start=(j == 0), stop=(j == 3))
            qt = small.tile([128, 1], F32, tag=f"qt{jo}")
            nc.scalar.activation(qt[:], qp[:],
                                 mybir.ActivationFunctionType.Identity,
                                 bias=bq_sb[jo][:], scale=1.0 / NPIX)
            qt_sb.append(qt)
        qsel = []
        for jo in range(4):
            qs = small.tile([128, 8], F32, tag=f"qsel{jo}")
            nc.vector.tensor_copy(qs[:], z32[:])
            nc.vector.tensor_copy(qs[0:64, 2 * jo:2 * jo + 1], qt_sb[jo][0:64, :])
            nc.vector.tensor_copy(qs[64:128, 2 * jo + 1:2 * jo + 2],
                                  qt_sb[jo][64:128, :])
            qsel.append(qs)
        wqk = []
        for j in range(4):
            wp = ps_sc.tile([128, 8], F32, tag="scd")
            for jo in range(4):
                nc.tensor.matmul(wp[:], wkT[jo][:, j * 128:(j + 1) * 128],
                                 qsel[jo][:], start=(jo == 0), stop=(jo == 3))
            wq_t = small.tile([128, 8], BF16, tag=f"wqk{j}")
            nc.vector.tensor_copy(wq_t[:], wp[:])
            wqk.append(wq_t)
        return wqk

    def phase_attend(b, wqk):
        """Merged scores + V + wt + nhv/dps, staggered by one e-group."""
        LAG = 4
        nhv = ps_nhv.tile([128, 512], F32, tag="nhv")
        scd = ps_sc.tile([128, 40], F32, tag="scd")
        dsum = small.tile([128, 8], F32, name=f"dsum{b}", tag=f"dsum{b}")
        e_groups = [None] * (NTILES // 4)
        for t in range(NTILES + LAG):
            if t < NTILES:
                g, k = t // 4, t % 4
                for j in range(4):
                    nc.tensor.matmul(scd[:, k * 8:(k + 1) * 8],
                                     xT[b][:, j, t * 128:(t + 1) * 128],
                                     wqk[j][:], start=(j == 0), stop=(j == 3))
                if k == 3:
                    eg = epool.tile([128, 32], BF16, tag="e")
                    nc.scalar.activation(eg[:], scd[:, 0:32],
                                         mybir.ActivationFunctionType.Exp,
                                         scale=1.0 / np.sqrt(DK))
                    e_groups[g] = eg
                    # per-group closed dps accumulation, drained to SBUF
                    for kk in range(4):
                        nc.tensor.matmul(scd[:, 32:40],
                                         msk_sb[:, 4 * g + kk, :],
                                         eg[:, kk * 8:(kk + 1) * 8],
                                         start=(kk == 0), stop=(kk == 3))
                    if g == 0:
                        nc.vector.tensor_copy(dsum[:], scd[:, 32:40])
                    else:
                        nc.vector.tensor_add(dsum[:], dsum[:], scd[:, 32:40])
            if t >= LAG:
                tv = t - LAG
                eg = e_groups[tv // 4]
                eslice = eg[:, (tv % 4) * 8:(tv % 4) * 8 + 8]
                vp = ps_v.tile([128, 512], F32, tag="v")
                for j in range(4):
                    nc.tensor.matmul(vp[:], xT[b][:, j, tv * 128:(tv + 1) * 128],
                                     wv_sb[j][:], start=(j == 0), stop=(j == 3))
                wt = wpool.tile([128, 512], BF16, tag="w")
                nc.vector.tensor_tensor(
                    wt[:].rearrange("p (n v) -> p n v", n=8),
                    vp[:].rearrange("p (n v) -> p n v", n=8),
                    eslice.unsqueeze(2).broadcast_to([128, 8, 64]),
                    op=mybir.AluOpType.mult)
                nc.tensor.matmul(nhv[:], msk_sb[:, tv, :], wt[:],
                                 start=(tv == 0), stop=(tv == NTILES - 1))
        return nhv, dsum

    def phase_norm(b, nhv, dps):
        """A_h, A_v = numerators/denominators + bv; transpose to [nv, h|w]."""
        dr = small.tile([128, 8], F32, tag="dr")
        nc.vector.reciprocal(dr[:], dps[:])  # dps: SBUF dsum
        ah = att.tile([64, 512], F32R, tag="ah")
        av = att.tile([64, 512], F32R, tag="av")
        nc.vector.tensor_tensor(
            ah[:].rearrange("p (n v) -> p n v", n=8),
            nhv[0:64, :].rearrange("p (n v) -> p n v", n=8),
            dr[0:64, :].unsqueeze(2).broadcast_to([64, 8, 64]),
            op=mybir.AluOpType.mult)
        nc.vector.tensor_add(ah[:], ah[:], bv_rep[:])
        nc.vector.tensor_tensor(
            av[:].rearrange("p (n v) -> p n v", n=8),
            nhv[64:128, :].rearrange("p (n v) -> p n v", n=8),
            dr[64:128, :].unsqueeze(2).broadcast_to([64, 8, 64]),
            op=mybir.AluOpType.mult)
        nc.vector.tensor_add(av[:], av[:], bv_rep[:])

        ahT, avT = [], []
        for j in range(4):
            tp = ps_tp.tile([128, 2, 64], F32, tag="tp")
            nc.tensor.matmul(tp[:, 0, :], ah[:, j * 128:(j + 1) * 128],
                             id64r[:], start=True, stop=True)
            nc.tensor.matmul(tp[:, 1, :], av[:, j * 128:(j + 1) * 128],
                             id64r[:], start=True, stop=True)
            t_sb = att.tile([128, 2, 64], F32, tag=f"ahvT{j}")
            nc.vector.tensor_copy(t_sb[:], tp[:])
            ahT.append(t_sb[:, 0, :])
            avT.append(t_sb[:, 1, :])
        return ahT, avT

    def phase_out_unit(b, g, ahT, avT):
        """One block g: combine outer product + output projection + DMA."""
        atp = ps_big.tile([64, 512], F32, tag="big")
        for j in range(4):
            pt = ppool.tile([128, 512], F32R, tag="p")
            eng = nc.gpsimd if j % 2 == 1 else nc.vector
            eng.tensor_tensor(
                pt[:].rearrange("p (h w) -> p h w", h=8),
                ahT[j][:, g * 8:(g + 1) * 8].unsqueeze(2)
                    .broadcast_to([128, 8, 64]),
                avT[j][:].unsqueeze(1).broadcast_to([128, 8, 64]),
                op=mybir.AluOpType.mult)
            nc.tensor.matmul(atp[:], iir[:], pt[:],
                             start=(j == 0), stop=(j == 3))
        at_sb = at_tiles[g % 2]
        nc.scalar.activation(at_sb[0:64, :], atp[:],
                             mybir.ActivationFunctionType.Copy)
        ot4 = wpool.tile([128, 4, 512], BF16, tag="ow")
        for tt in range(4):
            opool, optag = [(ps_big, "big"), (ps_v, "v")][tt % 2]
            op_ = opool.tile([128, 512], F32, tag=optag)
            nc.tensor.matmul(op_[:], at_sb[:, tt * 128:(tt + 1) * 128],
                             woe_sb[:], start=True, stop=True)
            if tt % 2 == 0:
                nc.scalar.activation(ot4[:, tt, :], op_[:],
                                     mybir.ActivationFunctionType.Copy)
            else:
                nc.vector.tensor_copy(ot4[:, tt, :], op_[:])
        nc.sync.dma_start(
            out_d[b, g * 512:(g + 1) * 512, :]
            .rearrange("(t p) c -> p t c", p=128), ot4[:])

    # ---------------- software-pipelined emission ----------------
    xs0 = phase_load(0, _prefetch0)
    wqk0 = phase_q(0, xs0)
    nhv0, dps0 = phase_attend(0, wqk0)
    ahT0, avT0 = phase_norm(0, nhv0, dps0)
    # interleave image 0 output with image 1 load
    xs1 = small.tile([128, 4], F32, name="xs1", tag="xs1")
    for g in range(NBLK):
        phase_out_unit(0, g, ahT0, avT0)
        load_macro(1, g, xs1)
    wqk1 = phase_q(1, xs1)
    nhv1, dps1 = phase_attend(1, wqk1)
    ahT1, avT1 = phase_norm(1, nhv1, dps1)
    for g in range(NBLK):
        phase_out_unit(1, g, ahT1, avT1)

    ctx.close()


_NC_CACHE = None
PROFILE = False
PROFILE_DIR = None


def kernel(**inputs):
    global _NC_CACHE
    x = np.asarray(inputs["x"], dtype=np.float32)
    Wq = np.asarray(inputs["Wq"], dtype=np.float32)
    bq = np.asarray(inputs["bq"], dtype=np.float32)
    Wk = np.asarray(inputs["Wk"], dtype=np.float32)
    bk = np.asarray(inputs["bk"], dtype=np.float32)
    Wv = np.asarray(inputs["Wv"], dtype=np.float32)
    bv = np.asarray(inputs["bv"], dtype=np.float32)
    Wo = np.asarray(inputs["Wo"], dtype=np.float32)
    bo = np.asarray(inputs["bo"], dtype=np.float32)

    if _NC_CACHE is None:
        _NC_CACHE = _build_kernel()
    nc = _NC_CACHE

    woe = np.concatenate([Wo, bo[None, :]], axis=0)
    idext = np.zeros((128, 129), dtype=BF)
    idext[:, 0:128] = np.eye(128, dtype=BF)
    idext[:, 128] = 1
    ii64 = np.tile(np.eye(64, dtype=BF), (2, 1))
    masks = np.zeros((NTILES, 128, 128), dtype=BF)
    for t in range(NTILES):
        masks[t, 0:64, 2 * t] = 1.0        # Sel_h: h == 2t for first h-row
        masks[t, 64:128, 2 * t + 1] = 1.0  # Sel_h: h == 2t+1 for second
        masks[t, :, 64:128] = np.tile(np.eye(64, dtype=BF), (2, 1))
    xbf = x.astype(BF)
    shared = dict(Wq=Wq, WkT=np.ascontiguousarray(Wk.T), Wv=Wv, Wo_ext=woe,
                  bq=bq, bv=bv, idext=idext, ii64=ii64, masks=masks)
    in_maps = []
    for c in range(NCORES):
        m = {"x": xbf[c * BPC:(c + 1) * BPC].reshape(BPC, NPIX, C).copy()}
        m.update(shared)
        in_maps.append(m)

    res = bass_utils.run_bass_kernel_spmd(nc, in_maps, core_ids=list(range(NCORES)),
                                          trace=PROFILE, tmpdir=PROFILE_DIR)
    if PROFILE:
        print("HW exec time:", res.exec_time_ns, "ns")
    outs = [np.asarray(res.results[c]["out"], dtype=np.float32)
            .reshape(BPC, H, W, DO) for c in range(NCORES)]
    return np.concatenate(outs, axis=0)


if __name__ == "__main__":
    rng = np.random.default_rng(0)
    ins = {
        "x": rng.standard_normal((B, H, W, C), dtype=np.float32),
        "Wq": rng.standard_normal((C, 512), dtype=np.float32) * 0.04,
        "bq": np.zeros(512, np.float32),
        "Wk": rng.standard_normal((C, 512), dtype=np.float32) * 0.04,
        "bk": np.zeros(512, np.float32),
        "Wv": rng.standard_normal((C, 512), dtype=np.float32) * 0.04,
        "bv": np.zeros(512, np.float32),
        "Wo": rng.standard_normal((64, 512), dtype=np.float32) * 0.1,
        "bo": np.zeros(512, np.float32),
    }
    out = kernel(**ins)
    print("kernel output", out.shape, out.dtype)


# revision 20
# speedup vs baseline: 1.0269x; 1.0269x over previous
"""Trainium2 Bass kernel for nn_AttentionModule (sparse axial-pooled attention).

Strategy: data-parallel over batch B=16 across 8 NeuronCores (2 images per
core), one SPMD program, no collectives.

Per image (H*W = 4096 pixels, C = 512):
  1. x arrives bf16; PE "transposes" each [128pix,128c] chunk via a plain
     matmul against an extended identity [I128 | 1]: out col 128 carries the
     per-chunk pixel-sum, so xsum rides the transpose for free.
  2. q = (xsum/4096) @ Wq + bq; wqk = fold of Wk with q (host passes Wk^T).
  3. Scores in natural pixel-major layout: s[pix, n] via stationary-xT
     matmuls with tiny free-8 outputs, exp'd in groups of 4 tiles on Act.
  4. V = xT @ Wv (f32r weights, full PE rate); wt = E * V elementwise
     (DVE/Pool); masked-sum matmuls give softmax numerators/denominators.
  5. A_h/A_v normalize, transpose (plain-mm vs bf16 identity), combine via
     broadcast products (DVE/Pool) + stacked-identity pair-sum matmul.
  6. out = A @ [Wo; bo]; result DMA'd straight from PSUM to DRAM.
"""

import sys

sys.path.insert(0, "/opt/trn_rl_repo")

import numpy as np
import ml_dtypes

import concourse.bass as bass
import concourse.tile as tile
from concourse import bacc, mybir
from concourse import bass_utils

F32 = mybir.dt.float32
F32R = mybir.dt.float32r
BF16 = mybir.dt.bfloat16
BF = ml_dtypes.bfloat16

B, H, W, C = 16, 64, 64, 512
NHEAD, DK, DV, DO = 8, 64, 64, 512
NCORES = 8
BPC = B // NCORES          # images per core
NPIX = H * W               # 4096
NTILES = NPIX // 128       # 32 pixel tiles per image
NBLK = NPIX // 512         # 8 pixel blocks per image


def _build_kernel():
    nc = bacc.Bacc("TRN2", target_bir_lowering=False, debug=False)

    dram = {}
    def din(name, shape, dt=F32):
        dram[name] = nc.dram_tensor(name, list(shape), dt, kind="ExternalInput").ap()
        return dram[name]

    x_d = din("x", (BPC, NPIX, C), BF16)
    wq_d = din("Wq", (C, NHEAD * DK))
    wkT_d = din("WkT", (NHEAD * DK, C))
    wv_d = din("Wv", (C, NHEAD * DV))
    woe_d = din("Wo_ext", (DV + 1, DO))      # [Wo; bo]
    bq_d = din("bq", (NHEAD * DK,))
    bv_d = din("bv", (NHEAD * DV,))
    idx_d = din("idext", (128, 129), BF16)   # [I128 | ones] for transpose+sum
    ii_d = din("ii64", (128, 64), BF16)      # two stacked 64-identities
    msk_d = din("masks", (NTILES, 128, 128), BF16)

    out_d = nc.dram_tensor("out", [BPC, NPIX, DO], BF16, kind="ExternalOutput").ap()

    with tile.TileContext(nc) as tc:
        _body(tc, x_d, wq_d, wkT_d, wv_d, woe_d, bq_d, bv_d,
              idx_d, ii_d, msk_d, out_d)

    nc.compile()
    return nc


def _body(tc, x_d, wq_d, wkT_d, wv_d, woe_d, bq_d, bv_d,
          idx_d, ii_d, msk_d, out_d):
    nc = tc.nc
    from contextlib import ExitStack
    ctx = ExitStack()

    const = ctx.enter_context(tc.tile_pool(name="const", bufs=1))
    xtp = ctx.enter_context(tc.tile_pool(name="xtp", bufs=1))
    xload = ctx.enter_context(tc.tile_pool(name="xload", bufs=8))
    epool = ctx.enter_context(tc.tile_pool(name="epool", bufs=10))
    wpool = ctx.enter_context(tc.tile_pool(name="wpool", bufs=4))
    small = ctx.enter_context(tc.tile_pool(name="small", bufs=2))
    att = ctx.enter_context(tc.tile_pool(name="att", bufs=2))
    ppool = ctx.enter_context(tc.tile_pool(name="ppool", bufs=6))
    atpool = ctx.enter_context(tc.tile_pool(name="atpool", bufs=1))

    # PSUM: 8 banks.  tp(2) + sc(1) + v(1) + nhv(1) + big(3) = 8
    ps_tp = ctx.enter_context(tc.tile_pool(name="ps_tp", bufs=2, space="PSUM"))
    ps_sc = ctx.enter_context(tc.tile_pool(name="ps_sc", bufs=1, space="PSUM"))
    ps_v = ctx.enter_context(tc.tile_pool(name="ps_v", bufs=2, space="PSUM"))
    ps_nhv = ctx.enter_context(tc.tile_pool(name="ps_nhv", bufs=1, space="PSUM"))
    ps_big = ctx.enter_context(tc.tile_pool(name="ps_big", bufs=2, space="PSUM"))

    def issue_x_dma_early(g):
        xt = xload.tile([128, 4, 512], BF16, tag="xt")
        nc.sync.dma_start(
            xt[:], x_d[0, g * 512:(g + 1) * 512, :]
            .rearrange("(t p) c -> p t c", p=128))
        return xt

    # ---- constants into SBUF (once per core); x DMAs are emitted first
    # in phase_load, so only idext goes ahead of them ----
    idx_sb = const.tile([128, 129], BF16, tag="idx")
    nc.sync.dma_start(idx_sb[:], idx_d)
    ii_sb = const.tile([128, 64], BF16, tag="ii")
    nc.sync.dma_start(ii_sb[:], ii_d)

    _prefetch0 = [issue_x_dma_early(g) for g in range(NBLK)]

    def load_r(shape, tag, src):
        stage = xload.tile(list(shape), F32, tag="wstage", bufs=1)
        nc.sync.dma_start(stage[:], src)
        t = const.tile(list(shape), F32R, tag=tag)
        nc.vector.tensor_copy(t[:], stage[:])
        return t

    wvstage = xload.tile([128, 4, 512], F32, name="wvstage", tag="wstage", bufs=1)
    _dummy = None
    nc.sync.dma_start(wvstage[:], wv_d.rearrange("(j p) c -> p j c", p=128))
    wv4 = const.tile([128, 4, 512], BF16, name="wv4", tag="wv4")
    nc.vector.tensor_copy(wv4[:], wvstage[:])
    wv_sb = [wv4[:, j, :] for j in range(4)]
    woe_sb = load_r([DV + 1, DO], "woe", woe_d)

    msk_sb = const.tile([128, NTILES, 128], BF16, name="msk", tag="msk")
    nc.sync.dma_start(msk_sb[:], msk_d.transpose([1, 0, 2]))

    wq4 = const.tile([128, 4, 512], F32, name="wq4", tag="wq4")
    nc.sync.dma_start(wq4[:], wq_d.rearrange("(j p) c -> p j c", p=128))
    wkT4 = const.tile([128, 4, 512], F32, name="wkT4", tag="wkT4")
    nc.sync.dma_start(wkT4[:], wkT_d.rearrange("(j p) c -> p j c", p=128))
    wq_sb = [wq4[:, j, :] for j in range(4)]
    wkT = [wkT4[:, j, :] for j in range(4)]
    bq4 = const.tile([128, 4], F32, name="bq4", tag="bq4")
    nc.sync.dma_start(bq4[:], bq_d.rearrange("(j p) -> p j", p=128))
    bq_sb = [bq4[:, j:j + 1] for j in range(4)]

    bv0 = const.tile([1, 512], F32, tag="bv0")
    nc.sync.dma_start(bv0[:], bv_d.unsqueeze(0))
    bv_rep = const.tile([64, 512], F32, tag="bvrep")
    nc.gpsimd.partition_broadcast(bv_rep[:], bv0[:])

    id64r = const.tile([64, 64], F32R, tag="id64r")
    nc.vector.tensor_copy(id64r[:], idx_sb[0:64, 0:64])
    iir = const.tile([128, 64], F32R, tag="iir")
    nc.vector.tensor_copy(iir[:], ii_sb[:])
    z32 = const.tile([128, 8], F32, tag="z32")
    nc.vector.memset(z32[:], 0.0)
    ones32 = const.tile([1, 512], F32, tag="ones32")
    nc.vector.memset(ones32[:], 1.0)

    # persistent per-image state
    xT = [xtp.tile([128, 4, NPIX], BF16, name=f"xT{b}", tag=f"xT{b}")
          for b in range(BPC)]

    # at-tiles: ones row written once, [0:64] rewritten per block
    at_tiles = [atpool.tile([DV + 1, 512], F32R, name=f"at{i}", tag=f"at{i}")
                for i in range(2)]
    for i in range(2):
        nc.scalar.activation(at_tiles[i][64:65, :], ones32[:],
                             mybir.ActivationFunctionType.Copy)

    # ---------------- per-image phases ----------------
    def issue_x_dma(b, g):
        xt = xload.tile([128, 4, 512], BF16, tag="xt")
        nc.sync.dma_start(
            xt[:], x_d[b, g * 512:(g + 1) * 512, :]
            .rearrange("(t p) c -> p t c", p=128))
        return xt

    def load_macro(b, g, xs32, xt=None):
        """One 4-pixel-tile macro: transposes + evac + pixel-sums.
        Rotate transpose psum over pools idle in this phase."""
        if xt is None:
            xt = issue_x_dma(b, g)
        if b == 0:
            rot = [(ps_tp, "tp"), (ps_tp, "tp"), (ps_v, "v"), (ps_v, "v"),
                   (ps_sc, "scd"), (ps_nhv, "nhv")]
        else:
            rot = [(ps_tp, "tp"), (ps_tp, "tp"),
                   (ps_sc, "scd"), (ps_nhv, "nhv")]
        ri = 0
        for t in range(4 * g, 4 * g + 4):
            for half in range(2):
                pool, ptag = rot[(8 * (g % len(rot)) + ri) % len(rot)]
                ri += 1
                ps = pool.tile([128, 2, 129], F32, tag=ptag)
                for j in range(2):
                    jj = half * 2 + j
                    nc.tensor.matmul(ps[:, j, :],
                                     xt[:, t % 4, jj * 128:(jj + 1) * 128],
                                     idx_sb[:], start=True, stop=True)
                dst = xT[b][:, half * 2:half * 2 + 2, t * 128:(t + 1) * 128]
                srcv = ps[:, :, 0:128]
                if (2 * t + half) % 4 != 3:
                    nc.scalar.activation(dst, srcv,
                                         mybir.ActivationFunctionType.Copy)
                else:
                    nc.vector.tensor_copy(dst, srcv)
                sums = ps[:, :, 128:129]
                v = xs32[:, half * 2:half * 2 + 2].unsqueeze(2)
                if t == 0:
                    nc.vector.tensor_copy(v, sums)
                else:
                    nc.vector.tensor_add(v, v, sums)

    def phase_load(b, prefetched=None):
        xs32 = small.tile([128, 4], F32, name=f"xs{b}", tag=f"xs{b}")
        for g in range(NBLK):
            load_macro(b, g, xs32,
                       prefetched[g] if prefetched is not None else None)
        return xs32

    def phase_q(b, xs32):
        """q = (xsum/4096) @ Wq + bq; wqk[c, n] fold (bf16)."""
        qt_sb = []
        for jo in range(4):
            qp = ps_sc.tile([128, 1], F32, tag="scd")
            for j in range(4):
                nc.tensor.matmul(qp[:],
                                 wq_sb[j][:, jo * 128:(jo + 1) * 128],
                                 xs32[:, j:j + 1],
                                 start=(j == 0), stop=(j == 3))
            qt = small.tile([128, 1], F32, tag=f"qt{jo}")
            nc.scalar.activation(qt[:], qp[:],
                                 mybir.ActivationFunctionType.Identity,
                                 bias=bq_sb[jo][:], scale=1.0 / NPIX)
            qt_sb.append(qt)
        qsel = []
        for jo in range(4):
            qs = small.tile([128, 8], F32, tag=f"qsel{jo}")
            nc.vector.tensor_copy(qs[:], z32[:])
            nc.vector.tensor_copy(qs[0:64, 2 * jo:2 * jo + 1], qt_sb[jo][0:64, :])
            nc.vector.tensor_copy(qs[64:128, 2 * jo + 1:2 * jo + 2],
                                  qt_sb[jo][64:128, :])
            qsel.append(qs)
        wqk = []
        for j in range(4):
            wp = ps_sc.tile([128, 8], F32, tag="scd")
            for jo in range(4):
                nc.tensor.matmul(wp[:], wkT[jo][:, j * 128:(j + 1) * 128],
                                 qsel[jo][:], start=(jo == 0), stop=(jo == 3))
            wq_t = small.tile([128, 8], BF16, tag=f"wqk{j}")
            nc.vector.tensor_copy(wq_t[:], wp[:])
            wqk.append(wq_t)
        return wqk

    def phase_attend(b, wqk):
        """Merged scores + V + wt + nhv/dps, staggered by one e-group."""
        LAG = 4
        nhv = ps_nhv.tile([128, 512], F32, tag="nhv")
        scd = ps_sc.tile([128, 40], F32, tag="scd")
        dsum = small.tile([128, 8], F32, name=f"dsum{b}", tag=f"dsum{b}")
        e_groups = [None] * (NTILES // 4)
        for t in range(NTILES + LAG):
            if t < NTILES:
                g, k = t // 4, t % 4
                for j in range(4):
                    nc.tensor.matmul(scd[:, k * 8:(k + 1) * 8],
                                     xT[b][:, j, t * 128:(t + 1) * 128],
                                     wqk[j][:], start=(j == 0), stop=(j == 3))
                if k == 3:
                    eg = epool.tile([128, 32], BF16, tag="e")
                    nc.scalar.activation(eg[:], scd[:, 0:32],
                                         mybir.ActivationFunctionType.Exp,
                                         scale=1.0 / np.sqrt(DK))
                    e_groups[g] = eg
                    # per-group closed dps accumulation, drained to SBUF
                    for kk in range(4):
                        nc.tensor.matmul(scd[:, 32:40],
                                         msk_sb[:, 4 * g + kk, :],
                                         eg[:, kk * 8:(kk + 1) * 8],
                                         start=(kk == 0), stop=(kk == 3))
                    if g == 0:
                        nc.vector.tensor_copy(dsum[:], scd[:, 32:40])
                    else:
                        nc.vector.tensor_add(dsum[:], dsum[:], scd[:, 32:40])
            if t >= LAG:
                tv = t - LAG
                eg = e_groups[tv // 4]
                eslice = eg[:, (tv % 4) * 8:(tv % 4) * 8 + 8]
                vp = ps_v.tile([128, 512], F32, tag="v")
                for j in range(4):
                    nc.tensor.matmul(vp[:], xT[b][:, j, tv * 128:(tv + 1) * 128],
                                     wv_sb[j][:], start=(j == 0), stop=(j == 3))
                wt = wpool.tile([128, 512], BF16, tag="w")
                nc.vector.tensor_tensor(
                    wt[:].rearrange("p (n v) -> p n v", n=8),
                    vp[:].rearrange("p (n v) -> p n v", n=8),
                    eslice.unsqueeze(2).broadcast_to([128, 8, 64]),
                    op=mybir.AluOpType.mult)
                nc.tensor.matmul(nhv[:], msk_sb[:, tv, :], wt[:],
                                 start=(tv == 0), stop=(tv == NTILES - 1))
        return nhv, dsum

    def phase_norm(b, nhv, dps):
        """A_h, A_v = numerators/denominators + bv; transpose to [nv, h|w]."""
        dr = small.tile([128, 8], F32, tag="dr")
        nc.vector.reciprocal(dr[:], dps[:])  # dps: SBUF dsum
        ah = att.tile([64, 512], F32R, tag="ah")
        av = att.tile([64, 512], F32R, tag="av")
        nc.vector.tensor_tensor(
            ah[:].rearrange("p (n v) -> p n v", n=8),
            nhv[0:64, :].rearrange("p (n v) -> p n v", n=8),
            dr[0:64, :].unsqueeze(2).broadcast_to([64, 8, 64]),
            op=mybir.AluOpType.mult)
        nc.vector.tensor_add(ah[:], ah[:], bv_rep[:])
        nc.vector.tensor_tensor(
            av[:].rearrange("p (n v) -> p n v", n=8),
            nhv[64:128, :].rearrange("p (n v) -> p n v", n=8),
            dr[64:128, :].unsqueeze(2).broadcast_to([64, 8, 64]),
            op=mybir.AluOpType.mult)
        nc.vector.tensor_add(av[:], av[:], bv_rep[:])

        ahT, avT = [], []
        for j in range(4):
            tp = ps_tp.tile([128, 2, 64], F32, tag="tp")
            nc.tensor.matmul(tp[:, 0, :], ah[:, j * 128:(j + 1) * 128],
                             id64r[:], start=True, stop=True)
            nc.tensor.matmul(tp[:, 1, :], av[:, j * 128:(j + 1) * 128],
                             id64r[:], start=True, stop=True)
            t_sb = att.tile([128, 2, 64], F32, tag=f"ahvT{j}")
            nc.vector.tensor_copy(t_sb[:], tp[:])
            ahT.append(t_sb[:, 0, :])
            avT.append(t_sb[:, 1, :])
        return ahT, avT

    def phase_out_unit(b, g, ahT, avT):
        """One block g: combine outer product + output projection + DMA."""
        atp = ps_big.tile([64, 512], F32, tag="big")
        for j in range(4):
            pt = ppool.tile([128, 512], F32R, tag="p")
            eng = nc.gpsimd if j % 2 == 1 else nc.vector
            eng.tensor_tensor(
                pt[:].rearrange("p (h w) -> p h w", h=8),
                ahT[j][:, g * 8:(g + 1) * 8].unsqueeze(2)
                    .broadcast_to([128, 8, 64]),
                avT[j][:].unsqueeze(1).broadcast_to([128, 8, 64]),
                op=mybir.AluOpType.mult)
            nc.tensor.matmul(atp[:], iir[:], pt[:],
                             start=(j == 0), stop=(j == 3))
        at_sb = at_tiles[g % 2]
        nc.scalar.activation(at_sb[0:64, :], atp[:],
                             mybir.ActivationFunctionType.Copy)
        ot4 = wpool.tile([128, 4, 512], BF16, tag="ow")
        for tt in range(4):
            opool, optag = [(ps_big, "big"), (ps_v, "v")][tt % 2]
            op_ = opool.tile([128, 512], F32, tag=optag)
            nc.tensor.matmul(op_[:], at_sb[:, tt * 128:(tt + 1) * 128],
                             woe_sb[:], start=True, stop=True)
            if tt % 2 == 0:
                nc.scalar.activation(ot4[:, tt, :], op_[:],
                                     mybir.ActivationFunctionType.Copy)
            else:
                nc.vector.tensor_copy(ot4[:, tt, :], op_[:])
        nc.sync.dma_start(
            out_d[b, g * 512:(g + 1) * 512, :]
            .rearrange("(t p) c -> p t c", p=128), ot4[:])

    # ---------------- software-pipelined emission ----------------
    xs0 = phase_load(0, _prefetch0)
    wqk0 = phase_q(0, xs0)
    nhv0, dps0 = phase_attend(0, wqk0)
    ahT0, avT0 = phase_norm(0, nhv0, dps0)
    # interleave image 0 output with image 1 load
    xs1 = small.tile([128, 4], F32, name="xs1", tag="xs1")
    for g in range(NBLK):
        phase_out_unit(0, g, ahT0, avT0)
        load_macro(1, g, xs1)
    wqk1 = phase_q(1, xs1)
    nhv1, dps1 = phase_attend(1, wqk1)
    ahT1, avT1 = phase_norm(1, nhv1, dps1)
    for g in range(NBLK):
        phase_out_unit(1, g, ahT1, avT1)

    ctx.close()


_NC_CACHE = None
PROFILE = False
PROFILE_DIR = None


def kernel(**inputs):
    global _NC_CACHE
    x = np.asarray(inputs["x"], dtype=np.float32)
    Wq = np.asarray(inputs["Wq"], dtype=np.float32)
    bq = np.asarray(inputs["bq"], dtype=np.float32)
    Wk = np.asarray(inputs["Wk"], dtype=np.float32)
    bk = np.asarray(inputs["bk"], dtype=np.float32)
    Wv = np.asarray(inputs["Wv"], dtype=np.float32)
    bv = np.asarray(inputs["bv"], dtype=np.float32)
    Wo = np.asarray(inputs["Wo"], dtype=np.float32)
    bo = np.asarray(inputs["bo"], dtype=np.float32)

    if _NC_CACHE is None:
        _NC_CACHE = _build_kernel()
    nc = _NC_CACHE

    woe = np.concatenate([Wo, bo[None, :]], axis=0)
    idext = np.zeros((128, 129), dtype=BF)
    idext[:, 0:128] = np.eye(128, dtype=BF)
    idext[:, 128] = 1
    ii64 = np.tile(np.eye(64, dtype=BF), (2, 1))
    masks = np.zeros((NTILES, 128, 128), dtype=BF)
    for t in range(NTILES):
        masks[t, 0:64, 2 * t] = 1.0        # Sel_h: h == 2t for first h-row
        masks[t, 64:128, 2 * t + 1] = 1.0  # Sel_h: h == 2t+1 for second
        masks[t, :, 64:128] = np.tile(np.eye(64, dtype=BF), (2, 1))
    xbf = x.astype(BF)
    shared = dict(Wq=Wq, WkT=np.ascontiguousarray(Wk.T), Wv=Wv, Wo_ext=woe,
                  bq=bq, bv=bv, idext=idext, ii64=ii64, masks=masks)
    in_maps = []
    for c in range(NCORES):
        m = {"x": xbf[c * BPC:(c + 1) * BPC].reshape(BPC, NPIX, C).copy()}
        m.update(shared)
        in_maps.append(m)

    res = bass_utils.run_bass_kernel_spmd(nc, in_maps, core_ids=list(range(NCORES)),
                                          trace=PROFILE, tmpdir=PROFILE_DIR)
    if PROFILE:
        print("HW exec time:", res.exec_time_ns, "ns")
    outs = [np.asarray(res.results[c]["out"], dtype=np.float32)
            .reshape(BPC, H, W, DO) for c in range(NCORES)]
    return np.concatenate(outs, axis=0)


if __name__ == "__main__":
    rng = np.random.default_rng(0)
    ins = {
        "x": rng.standard_normal((B, H, W, C), dtype=np.float32),
        "Wq": rng.standard_normal((C, 512), dtype=np.float32) * 0.04,
        "bq": np.zeros(512, np.float32),
        "Wk": rng.standard_normal((C, 512), dtype=np.float32) * 0.04,
        "bk": np.zeros(512, np.float32),
        "Wv": rng.standard_normal((C, 512), dtype=np.float32) * 0.04,
        "bv": np.zeros(512, np.float32),
        "Wo": rng.standard_normal((64, 512), dtype=np.float32) * 0.1,
        "bo": np.zeros(512, np.float32),
    }
    out = kernel(**ins)
    print("kernel output", out.shape, out.dtype)


# revision 21
# speedup vs baseline: 1.0366x; 1.0095x over previous
"""Trainium2 Bass kernel for nn_AttentionModule (sparse axial-pooled attention).

Strategy: data-parallel over batch B=16 across 8 NeuronCores (2 images per
core), one SPMD program, no collectives.

Per image (H*W = 4096 pixels, C = 512):
  1. x arrives bf16; PE "transposes" each [128pix,128c] chunk via a plain
     matmul against an extended identity [I128 | 1]: out col 128 carries the
     per-chunk pixel-sum, so xsum rides the transpose for free.
  2. q = (xsum/4096) @ Wq + bq; wqk = fold of Wk with q (host passes Wk^T).
  3. Scores in natural pixel-major layout: s[pix, n] via stationary-xT
     matmuls with tiny free-8 outputs, exp'd in groups of 4 tiles on Act.
  4. V = xT @ Wv (f32r weights, full PE rate); wt = E * V elementwise
     (DVE/Pool); masked-sum matmuls give softmax numerators/denominators.
  5. A_h/A_v normalize, transpose (plain-mm vs bf16 identity), combine via
     broadcast products (DVE/Pool) + stacked-identity pair-sum matmul.
  6. out = A @ [Wo; bo]; result DMA'd straight from PSUM to DRAM.
"""

import sys

sys.path.insert(0, "/opt/trn_rl_repo")

import numpy as np
import ml_dtypes

import concourse.bass as bass
import concourse.tile as tile
from concourse import bacc, mybir
from concourse import bass_utils

F32 = mybir.dt.float32
F32R = mybir.dt.float32r
BF16 = mybir.dt.bfloat16
BF = ml_dtypes.bfloat16

B, H, W, C = 16, 64, 64, 512
NHEAD, DK, DV, DO = 8, 64, 64, 512
NCORES = 8
BPC = B // NCORES          # images per core
NPIX = H * W               # 4096
NTILES = NPIX // 128       # 32 pixel tiles per image
NBLK = NPIX // 512         # 8 pixel blocks per image


def _build_kernel():
    nc = bacc.Bacc("TRN2", target_bir_lowering=False, debug=False)

    dram = {}
    def din(name, shape, dt=F32):
        dram[name] = nc.dram_tensor(name, list(shape), dt, kind="ExternalInput").ap()
        return dram[name]

    x_d = din("x", (BPC, NPIX, C), BF16)
    wq_d = din("Wq", (C, NHEAD * DK))
    wkT_d = din("WkT", (NHEAD * DK, C))
    wv_d = din("Wv", (C, NHEAD * DV))
    woe_d = din("Wo_ext", (DV + 1, DO))      # [Wo; bo]
    bq_d = din("bq", (NHEAD * DK,))
    bv_d = din("bv", (NHEAD * DV,))
    idx_d = din("idext", (128, 129), BF16)   # [I128 | ones] for transpose+sum
    ii_d = din("ii64", (128, 64), BF16)      # two stacked 64-identities
    msk_d = din("masks", (NTILES, 128, 128), BF16)

    out_d = nc.dram_tensor("out", [BPC, NPIX, DO], BF16, kind="ExternalOutput").ap()

    with tile.TileContext(nc) as tc:
        _body(tc, x_d, wq_d, wkT_d, wv_d, woe_d, bq_d, bv_d,
              idx_d, ii_d, msk_d, out_d)

    nc.compile()
    return nc


def _body(tc, x_d, wq_d, wkT_d, wv_d, woe_d, bq_d, bv_d,
          idx_d, ii_d, msk_d, out_d):
    nc = tc.nc
    from contextlib import ExitStack
    ctx = ExitStack()

    const = ctx.enter_context(tc.tile_pool(name="const", bufs=1))
    xtp = ctx.enter_context(tc.tile_pool(name="xtp", bufs=1))
    xload = ctx.enter_context(tc.tile_pool(name="xload", bufs=8))
    epool = ctx.enter_context(tc.tile_pool(name="epool", bufs=10))
    wpool = ctx.enter_context(tc.tile_pool(name="wpool", bufs=4))
    small = ctx.enter_context(tc.tile_pool(name="small", bufs=2))
    att = ctx.enter_context(tc.tile_pool(name="att", bufs=2))
    ppool = ctx.enter_context(tc.tile_pool(name="ppool", bufs=6))
    atpool = ctx.enter_context(tc.tile_pool(name="atpool", bufs=1))

    # PSUM: 8 banks.  tp(2) + sc(1) + v(1) + nhv(1) + big(3) = 8
    ps_tp = ctx.enter_context(tc.tile_pool(name="ps_tp", bufs=2, space="PSUM"))
    ps_sc = ctx.enter_context(tc.tile_pool(name="ps_sc", bufs=1, space="PSUM"))
    ps_v = ctx.enter_context(tc.tile_pool(name="ps_v", bufs=2, space="PSUM"))
    ps_nhv = ctx.enter_context(tc.tile_pool(name="ps_nhv", bufs=1, space="PSUM"))
    ps_big = ctx.enter_context(tc.tile_pool(name="ps_big", bufs=2, space="PSUM"))

    def issue_x_dma_early(g):
        xt = xload.tile([128, 4, 512], BF16, tag="xt")
        nc.sync.dma_start(
            xt[:], x_d[0, g * 512:(g + 1) * 512, :]
            .rearrange("(t p) c -> p t c", p=128))
        return xt

    # ---- constants into SBUF (once per core); x DMAs are emitted first
    # in phase_load, so only idext goes ahead of them ----
    idx_sb = const.tile([128, 129], BF16, tag="idx")
    nc.sync.dma_start(idx_sb[:], idx_d)
    ii_sb = const.tile([128, 64], BF16, tag="ii")
    nc.sync.dma_start(ii_sb[:], ii_d)

    _prefetch0 = [issue_x_dma_early(g) for g in range(NBLK)]

    def load_r(shape, tag, src):
        stage = xload.tile(list(shape), F32, tag="wstage", bufs=1)
        nc.sync.dma_start(stage[:], src)
        t = const.tile(list(shape), F32R, tag=tag)
        nc.vector.tensor_copy(t[:], stage[:])
        return t

    wvstage = xload.tile([128, 4, 512], F32, name="wvstage", tag="wstage", bufs=1)
    _dummy = None
    nc.sync.dma_start(wvstage[:], wv_d.rearrange("(j p) c -> p j c", p=128))
    wv4 = const.tile([128, 4, 512], BF16, name="wv4", tag="wv4")
    nc.vector.tensor_copy(wv4[:], wvstage[:])
    wv_sb = [wv4[:, j, :] for j in range(4)]
    woe_sb = load_r([DV + 1, DO], "woe", woe_d)

    msk_sb = const.tile([128, NTILES, 128], BF16, name="msk", tag="msk")
    nc.sync.dma_start(msk_sb[:], msk_d.transpose([1, 0, 2]))

    wq4 = const.tile([128, 4, 512], F32, name="wq4", tag="wq4")
    nc.sync.dma_start(wq4[:], wq_d.rearrange("(j p) c -> p j c", p=128))
    wkT4 = const.tile([128, 4, 512], F32, name="wkT4", tag="wkT4")
    nc.sync.dma_start(wkT4[:], wkT_d.rearrange("(j p) c -> p j c", p=128))
    wq_sb = [wq4[:, j, :] for j in range(4)]
    wkT = [wkT4[:, j, :] for j in range(4)]
    bq4 = const.tile([128, 4], F32, name="bq4", tag="bq4")
    nc.sync.dma_start(bq4[:], bq_d.rearrange("(j p) -> p j", p=128))
    bq_sb = [bq4[:, j:j + 1] for j in range(4)]

    bv0 = const.tile([1, 512], F32, tag="bv0")
    nc.sync.dma_start(bv0[:], bv_d.unsqueeze(0))
    bv_rep = const.tile([64, 512], F32, tag="bvrep")
    nc.gpsimd.partition_broadcast(bv_rep[:], bv0[:])

    id64r = const.tile([64, 64], F32R, tag="id64r")
    nc.vector.tensor_copy(id64r[:], idx_sb[0:64, 0:64])
    iir = const.tile([128, 64], F32R, tag="iir")
    nc.vector.tensor_copy(iir[:], ii_sb[:])
    z32 = const.tile([128, 8], F32, tag="z32")
    nc.vector.memset(z32[:], 0.0)
    ones32 = const.tile([1, 512], F32, tag="ones32")
    nc.vector.memset(ones32[:], 1.0)

    # persistent per-image state
    xT = [xtp.tile([128, 4, NPIX], BF16, name=f"xT{b}", tag=f"xT{b}")
          for b in range(BPC)]

    # at-tiles: ones row written once, [0:64] rewritten per block
    at_tiles = [atpool.tile([DV + 1, 512], F32R, name=f"at{i}", tag=f"at{i}")
                for i in range(2)]
    for i in range(2):
        nc.scalar.activation(at_tiles[i][64:65, :], ones32[:],
                             mybir.ActivationFunctionType.Copy)

    # ---------------- per-image phases ----------------
    def issue_x_dma(b, g):
        xt = xload.tile([128, 4, 512], BF16, tag="xt")
        nc.sync.dma_start(
            xt[:], x_d[b, g * 512:(g + 1) * 512, :]
            .rearrange("(t p) c -> p t c", p=128))
        return xt

    def load_macro(b, g, xs32, xt=None):
        """One 4-pixel-tile macro: transposes + evac + pixel-sums.
        Rotate transpose psum over pools idle in this phase."""
        if xt is None:
            xt = issue_x_dma(b, g)
        if b == 0:
            rot = [(ps_tp, "tp"), (ps_tp, "tp"), (ps_v, "v"), (ps_v, "v"),
                   (ps_sc, "scd"), (ps_nhv, "nhv")]
        else:
            rot = [(ps_tp, "tp"), (ps_tp, "tp"),
                   (ps_sc, "scd"), (ps_nhv, "nhv")]
        ri = 0
        for t in range(4 * g, 4 * g + 4):
            for half in range(2):
                pool, ptag = rot[(8 * (g % len(rot)) + ri) % len(rot)]
                ri += 1
                ps = pool.tile([128, 2, 129], F32, tag=ptag)
                for j in range(2):
                    jj = half * 2 + j
                    nc.tensor.matmul(ps[:, j, :],
                                     xt[:, t % 4, jj * 128:(jj + 1) * 128],
                                     idx_sb[:], start=True, stop=True)
                dst = xT[b][:, half * 2:half * 2 + 2, t * 128:(t + 1) * 128]
                srcv = ps[:, :, 0:128]
                if (2 * t + half) % 4 != 3:
                    nc.scalar.activation(dst, srcv,
                                         mybir.ActivationFunctionType.Copy)
                else:
                    nc.vector.tensor_copy(dst, srcv)
                sums = ps[:, :, 128:129]
                v = xs32[:, half * 2:half * 2 + 2].unsqueeze(2)
                if t == 0:
                    nc.vector.tensor_copy(v, sums)
                else:
                    nc.vector.tensor_add(v, v, sums)

    def phase_load(b, prefetched=None):
        xs32 = small.tile([128, 4], F32, name=f"xs{b}", tag=f"xs{b}")
        for g in range(NBLK):
            load_macro(b, g, xs32,
                       prefetched[g] if prefetched is not None else None)
        return xs32

    def phase_q(b, xs32):
        """q = (xsum/4096) @ Wq + bq; wqk[c, n] fold (bf16)."""
        qt_sb = []
        for jo in range(4):
            qp = ps_sc.tile([128, 1], F32, tag="scd")
            for j in range(4):
                nc.tensor.matmul(qp[:],
                                 wq_sb[j][:, jo * 128:(jo + 1) * 128],
                                 xs32[:, j:j + 1],
                                 start=(j == 0), stop=(j == 3))
            qt = small.tile([128, 1], F32, tag=f"qt{jo}")
            nc.scalar.activation(qt[:], qp[:],
                                 mybir.ActivationFunctionType.Identity,
                                 bias=bq_sb[jo][:], scale=1.0 / NPIX)
            qt_sb.append(qt)
        qsel = []
        for jo in range(4):
            qs = small.tile([128, 8], F32, tag=f"qsel{jo}")
            nc.vector.tensor_copy(qs[:], z32[:])
            nc.vector.tensor_copy(qs[0:64, 2 * jo:2 * jo + 1], qt_sb[jo][0:64, :])
            nc.vector.tensor_copy(qs[64:128, 2 * jo + 1:2 * jo + 2],
                                  qt_sb[jo][64:128, :])
            qsel.append(qs)
        wqk = []
        for j in range(4):
            wp = ps_sc.tile([128, 8], F32, tag="scd")
            for jo in range(4):
                nc.tensor.matmul(wp[:], wkT[jo][:, j * 128:(j + 1) * 128],
                                 qsel[jo][:], start=(jo == 0), stop=(jo == 3))
            wq_t = small.tile([128, 8], BF16, tag=f"wqk{j}")
            nc.vector.tensor_copy(wq_t[:], wp[:])
            wqk.append(wq_t)
        return wqk

    def phase_attend(b, wqk):
        """Merged scores + V + wt + nhv/dps, staggered by one e-group."""
        LAG = 4
        nhv = ps_nhv.tile([128, 512], F32, tag="nhv")
        dsum = small.tile([128, 8], F32, name=f"dsum{b}", tag=f"dsum{b}")
        e_groups = [None] * (NTILES // 4)
        scd = None
        for t in range(NTILES + LAG):
            if t < NTILES:
                g, k = t // 4, t % 4
                if k == 0:
                    gpool, gtag = [(ps_sc, "scd"), (ps_tp, "tp")][g % 2]
                    scd = gpool.tile([128, 40], F32, tag=gtag)
                for j in range(4):
                    nc.tensor.matmul(scd[:, k * 8:(k + 1) * 8],
                                     xT[b][:, j, t * 128:(t + 1) * 128],
                                     wqk[j][:], start=(j == 0), stop=(j == 3))
                if k == 3:
                    eg = epool.tile([128, 32], BF16, tag="e")
                    nc.scalar.activation(eg[:], scd[:, 0:32],
                                         mybir.ActivationFunctionType.Exp,
                                         scale=1.0 / np.sqrt(DK))
                    e_groups[g] = eg
                    # per-group closed dps accumulation, drained to SBUF
                    for kk in range(4):
                        nc.tensor.matmul(scd[:, 32:40],
                                         msk_sb[:, 4 * g + kk, :],
                                         eg[:, kk * 8:(kk + 1) * 8],
                                         start=(kk == 0), stop=(kk == 3))
                    if g == 0:
                        nc.vector.tensor_copy(dsum[:], scd[:, 32:40])
                    else:
                        nc.vector.tensor_add(dsum[:], dsum[:], scd[:, 32:40])
            if t >= LAG:
                tv = t - LAG
                eg = e_groups[tv // 4]
                eslice = eg[:, (tv % 4) * 8:(tv % 4) * 8 + 8]
                vp = ps_v.tile([128, 512], F32, tag="v")
                for j in range(4):
                    nc.tensor.matmul(vp[:], xT[b][:, j, tv * 128:(tv + 1) * 128],
                                     wv_sb[j][:], start=(j == 0), stop=(j == 3))
                wt = wpool.tile([128, 512], BF16, tag="w")
                nc.vector.tensor_tensor(
                    wt[:].rearrange("p (n v) -> p n v", n=8),
                    vp[:].rearrange("p (n v) -> p n v", n=8),
                    eslice.unsqueeze(2).broadcast_to([128, 8, 64]),
                    op=mybir.AluOpType.mult)
                nc.tensor.matmul(nhv[:], msk_sb[:, tv, :], wt[:],
                                 start=(tv == 0), stop=(tv == NTILES - 1))
        return nhv, dsum

    def phase_norm(b, nhv, dps):
        """A_h, A_v = numerators/denominators + bv; transpose to [nv, h|w]."""
        dr = small.tile([128, 8], F32, tag="dr")
        nc.vector.reciprocal(dr[:], dps[:])  # dps: SBUF dsum
        ah = att.tile([64, 512], F32R, tag="ah")
        av = att.tile([64, 512], F32R, tag="av")
        nc.vector.tensor_tensor(
            ah[:].rearrange("p (n v) -> p n v", n=8),
            nhv[0:64, :].rearrange("p (n v) -> p n v", n=8),
            dr[0:64, :].unsqueeze(2).broadcast_to([64, 8, 64]),
            op=mybir.AluOpType.mult)
        nc.vector.tensor_add(ah[:], ah[:], bv_rep[:])
        nc.vector.tensor_tensor(
            av[:].rearrange("p (n v) -> p n v", n=8),
            nhv[64:128, :].rearrange("p (n v) -> p n v", n=8),
            dr[64:128, :].unsqueeze(2).broadcast_to([64, 8, 64]),
            op=mybir.AluOpType.mult)
        nc.vector.tensor_add(av[:], av[:], bv_rep[:])

        ahT, avT = [], []
        for j in range(4):
            tp = ps_tp.tile([128, 2, 64], F32, tag="tp")
            nc.tensor.matmul(tp[:, 0, :], ah[:, j * 128:(j + 1) * 128],
                             id64r[:], start=True, stop=True)
            nc.tensor.matmul(tp[:, 1, :], av[:, j * 128:(j + 1) * 128],
                             id64r[:], start=True, stop=True)
            t_sb = att.tile([128, 2, 64], F32, tag=f"ahvT{j}")
            nc.vector.tensor_copy(t_sb[:], tp[:])
            ahT.append(t_sb[:, 0, :])
            avT.append(t_sb[:, 1, :])
        return ahT, avT

    def phase_out_unit(b, g, ahT, avT):
        """One block g: combine outer product + output projection + DMA."""
        atp = ps_big.tile([64, 512], F32, tag="big")
        for j in range(4):
            pt = ppool.tile([128, 512], F32R, tag="p")
            eng = nc.gpsimd if j % 2 == 1 else nc.vector
            eng.tensor_tensor(
                pt[:].rearrange("p (h w) -> p h w", h=8),
                ahT[j][:, g * 8:(g + 1) * 8].unsqueeze(2)
                    .broadcast_to([128, 8, 64]),
                avT[j][:].unsqueeze(1).broadcast_to([128, 8, 64]),
                op=mybir.AluOpType.mult)
            nc.tensor.matmul(atp[:], iir[:], pt[:],
                             start=(j == 0), stop=(j == 3))
        at_sb = at_tiles[g % 2]
        nc.scalar.activation(at_sb[0:64, :], atp[:],
                             mybir.ActivationFunctionType.Copy)
        ot4 = wpool.tile([128, 4, 512], BF16, tag="ow")
        for tt in range(4):
            opool, optag = [(ps_big, "big"), (ps_v, "v")][tt % 2]
            op_ = opool.tile([128, 512], F32, tag=optag)
            nc.tensor.matmul(op_[:], at_sb[:, tt * 128:(tt + 1) * 128],
                             woe_sb[:], start=True, stop=True)
            if tt % 2 == 0:
                nc.scalar.activation(ot4[:, tt, :], op_[:],
                                     mybir.ActivationFunctionType.Copy)
            else:
                nc.vector.tensor_copy(ot4[:, tt, :], op_[:])
        nc.sync.dma_start(
            out_d[b, g * 512:(g + 1) * 512, :]
            .rearrange("(t p) c -> p t c", p=128), ot4[:])

    # ---------------- software-pipelined emission ----------------
    xs0 = phase_load(0, _prefetch0)
    wqk0 = phase_q(0, xs0)
    nhv0, dps0 = phase_attend(0, wqk0)
    ahT0, avT0 = phase_norm(0, nhv0, dps0)
    # interleave image 0 output with image 1 load
    xs1 = small.tile([128, 4], F32, name="xs1", tag="xs1")
    for g in range(NBLK):
        phase_out_unit(0, g, ahT0, avT0)
        load_macro(1, g, xs1)
    wqk1 = phase_q(1, xs1)
    nhv1, dps1 = phase_attend(1, wqk1)
    ahT1, avT1 = phase_norm(1, nhv1, dps1)
    for g in range(NBLK):
        phase_out_unit(1, g, ahT1, avT1)

    ctx.close()


_NC_CACHE = None
PROFILE = False
PROFILE_DIR = None


def kernel(**inputs):
    global _NC_CACHE
    x = np.asarray(inputs["x"], dtype=np.float32)
    Wq = np.asarray(inputs["Wq"], dtype=np.float32)
    bq = np.asarray(inputs["bq"], dtype=np.float32)
    Wk = np.asarray(inputs["Wk"], dtype=np.float32)
    bk = np.asarray(inputs["bk"], dtype=np.float32)
    Wv = np.asarray(inputs["Wv"], dtype=np.float32)
    bv = np.asarray(inputs["bv"], dtype=np.float32)
    Wo = np.asarray(inputs["Wo"], dtype=np.float32)
    bo = np.asarray(inputs["bo"], dtype=np.float32)

    if _NC_CACHE is None:
        _NC_CACHE = _build_kernel()
    nc = _NC_CACHE

    woe = np.concatenate([Wo, bo[None, :]], axis=0)
    idext = np.zeros((128, 129), dtype=BF)
    idext[:, 0:128] = np.eye(128, dtype=BF)
    idext[:, 128] = 1
    ii64 = np.tile(np.eye(64, dtype=BF), (2, 1))
    masks = np.zeros((NTILES, 128, 128), dtype=BF)
    for t in range(NTILES):
        masks[t, 0:64, 2 * t] = 1.0        # Sel_h: h == 2t for first h-row
        masks[t, 64:128, 2 * t + 1] = 1.0  # Sel_h: h == 2t+1 for second
        masks[t, :, 64:128] = np.tile(np.eye(64, dtype=BF), (2, 1))
    xbf = x.astype(BF)
    shared = dict(Wq=Wq, WkT=np.ascontiguousarray(Wk.T), Wv=Wv, Wo_ext=woe,
                  bq=bq, bv=bv, idext=idext, ii64=ii64, masks=masks)
    in_maps = []
    for c in range(NCORES):
        m = {"x": xbf[c * BPC:(c + 1) * BPC].reshape(BPC, NPIX, C).copy()}
        m.update(shared)
        in_maps.append(m)

    res = bass_utils.run_bass_kernel_spmd(nc, in_maps, core_ids=list(range(NCORES)),
                                          trace=PROFILE, tmpdir=PROFILE_DIR)
    if PROFILE:
        print("HW exec time:", res.exec_time_ns, "ns")
    outs = [np.asarray(res.results[c]["out"], dtype=np.float32)
            .reshape(BPC, H, W, DO) for c in range(NCORES)]
    return np.concatenate(outs, axis=0)


if __name__ == "__main__":
    rng = np.random.default_rng(0)
    ins = {
        "x": rng.standard_normal((B, H, W, C), dtype=np.float32),
        "Wq": rng.standard_normal((C, 512), dtype=np.float32) * 0.04,
        "bq": np.zeros(512, np.float32),
        "Wk": rng.standard_normal((C, 512), dtype=np.float32) * 0.04,
        "bk": np.zeros(512, np.float32),
        "Wv": rng.standard_normal((C, 512), dtype=np.float32) * 0.04,
        "bv": np.zeros(512, np.float32),
        "Wo": rng.standard_normal((64, 512), dtype=np.float32) * 0.1,
        "bo": np.zeros(512, np.float32),
    }
    out = kernel(**ins)
    print("kernel output", out.shape, out.dtype)


# revision 22
# speedup vs baseline: 1.0611x; 1.0236x over previous
"""Trainium2 Bass kernel for nn_AttentionModule (sparse axial-pooled attention).

Strategy: data-parallel over batch B=16 across 8 NeuronCores (2 images per
core), one SPMD program, no collectives.

Per image (H*W = 4096 pixels, C = 512):
  1. x arrives bf16; PE "transposes" each [128pix,128c] chunk via a plain
     matmul against an extended identity [I128 | 1]: out col 128 carries the
     per-chunk pixel-sum, so xsum rides the transpose for free.
  2. q = (xsum/4096) @ Wq + bq; wqk = fold of Wk with q (host passes Wk^T).
  3. Scores in natural pixel-major layout: s[pix, n] via stationary-xT
     matmuls with tiny free-8 outputs, exp'd in groups of 4 tiles on Act.
  4. V = xT @ Wv (f32r weights, full PE rate); wt = E * V elementwise
     (DVE/Pool); masked-sum matmuls give softmax numerators/denominators.
  5. A_h/A_v normalize, transpose (plain-mm vs bf16 identity), combine via
     broadcast products (DVE/Pool) + stacked-identity pair-sum matmul.
  6. out = A @ [Wo; bo]; result DMA'd straight from PSUM to DRAM.
"""

import sys

sys.path.insert(0, "/opt/trn_rl_repo")

import numpy as np
import ml_dtypes

import concourse.bass as bass
import concourse.tile as tile
from concourse import bacc, mybir
from concourse import bass_utils

F32 = mybir.dt.float32
F32R = mybir.dt.float32r
BF16 = mybir.dt.bfloat16
BF = ml_dtypes.bfloat16

B, H, W, C = 16, 64, 64, 512
NHEAD, DK, DV, DO = 8, 64, 64, 512
NCORES = 8
BPC = B // NCORES          # images per core
NPIX = H * W               # 4096
NTILES = NPIX // 128       # 32 pixel tiles per image
NBLK = NPIX // 512         # 8 pixel blocks per image


def _build_kernel():
    nc = bacc.Bacc("TRN2", target_bir_lowering=False, debug=False)

    dram = {}
    def din(name, shape, dt=F32):
        dram[name] = nc.dram_tensor(name, list(shape), dt, kind="ExternalInput").ap()
        return dram[name]

    x_d = din("x", (BPC, NPIX, C), BF16)
    wq_d = din("Wq", (C, NHEAD * DK), BF16)
    wkT_d = din("WkT", (NHEAD * DK, C), BF16)
    wv_d = din("Wv", (C, NHEAD * DV), BF16)
    woe_d = din("Wo_ext", (DV + 1, DO))      # [Wo; bo]
    bq_d = din("bq", (NHEAD * DK,))
    bv_d = din("bv", (NHEAD * DV,))
    idx_d = din("idext", (128, 129), BF16)   # [I128 | ones] for transpose+sum
    ii_d = din("ii64", (128, 64), BF16)      # two stacked 64-identities
    msk_d = din("masks", (128, NTILES, 128), BF16)  # pre-transposed on host

    out_d = nc.dram_tensor("out", [BPC, NPIX, DO], BF16, kind="ExternalOutput").ap()

    with tile.TileContext(nc) as tc:
        _body(tc, x_d, wq_d, wkT_d, wv_d, woe_d, bq_d, bv_d,
              idx_d, ii_d, msk_d, out_d)

    nc.compile()
    return nc


def _body(tc, x_d, wq_d, wkT_d, wv_d, woe_d, bq_d, bv_d,
          idx_d, ii_d, msk_d, out_d):
    nc = tc.nc
    from contextlib import ExitStack
    ctx = ExitStack()

    const = ctx.enter_context(tc.tile_pool(name="const", bufs=1))
    xtp = ctx.enter_context(tc.tile_pool(name="xtp", bufs=1))
    xload = ctx.enter_context(tc.tile_pool(name="xload", bufs=8))
    epool = ctx.enter_context(tc.tile_pool(name="epool", bufs=10))
    wpool = ctx.enter_context(tc.tile_pool(name="wpool", bufs=4))
    small = ctx.enter_context(tc.tile_pool(name="small", bufs=2))
    att = ctx.enter_context(tc.tile_pool(name="att", bufs=2))
    ppool = ctx.enter_context(tc.tile_pool(name="ppool", bufs=6))
    atpool = ctx.enter_context(tc.tile_pool(name="atpool", bufs=1))

    # PSUM: 8 banks.  tp(2) + sc(1) + v(1) + nhv(1) + big(3) = 8
    ps_tp = ctx.enter_context(tc.tile_pool(name="ps_tp", bufs=2, space="PSUM"))
    ps_sc = ctx.enter_context(tc.tile_pool(name="ps_sc", bufs=1, space="PSUM"))
    ps_v = ctx.enter_context(tc.tile_pool(name="ps_v", bufs=2, space="PSUM"))
    ps_nhv = ctx.enter_context(tc.tile_pool(name="ps_nhv", bufs=1, space="PSUM"))
    ps_big = ctx.enter_context(tc.tile_pool(name="ps_big", bufs=2, space="PSUM"))

    def issue_x_dma_early(g):
        xt = xload.tile([128, 4, 512], BF16, tag="xt")
        nc.sync.dma_start(
            xt[:], x_d[0, g * 512:(g + 1) * 512, :]
            .rearrange("(t p) c -> p t c", p=128))
        return xt

    # ---- constants into SBUF (once per core); x DMAs are emitted first
    # in phase_load, so only idext goes ahead of them ----
    idx_sb = const.tile([128, 129], BF16, tag="idx")
    nc.sync.dma_start(idx_sb[:], idx_d)
    ii_sb = const.tile([128, 64], BF16, tag="ii")
    nc.sync.dma_start(ii_sb[:], ii_d)

    _prefetch0 = [issue_x_dma_early(g) for g in range(NBLK)]

    def load_r(shape, tag, src):
        stage = xload.tile(list(shape), F32, tag="wstage", bufs=1)
        nc.sync.dma_start(stage[:], src)
        t = const.tile(list(shape), F32R, tag=tag)
        nc.vector.tensor_copy(t[:], stage[:])
        return t

    wv4 = const.tile([128, 4, 512], BF16, name="wv4", tag="wv4")
    nc.sync.dma_start(wv4[:], wv_d.rearrange("(j p) c -> p j c", p=128))
    wv_sb = [wv4[:, j, :] for j in range(4)]
    woe_sb = load_r([DV + 1, DO], "woe", woe_d)

    msk_sb = const.tile([128, NTILES, 128], BF16, name="msk", tag="msk")
    nc.sync.dma_start(msk_sb[:], msk_d)

    wq4 = const.tile([128, 4, 512], BF16, name="wq4", tag="wq4")
    nc.sync.dma_start(wq4[:], wq_d.rearrange("(j p) c -> p j c", p=128))
    wkT4 = const.tile([128, 4, 512], BF16, name="wkT4", tag="wkT4")
    nc.sync.dma_start(wkT4[:], wkT_d.rearrange("(j p) c -> p j c", p=128))
    wq_sb = [wq4[:, j, :] for j in range(4)]
    wkT = [wkT4[:, j, :] for j in range(4)]
    bq4 = const.tile([128, 4], F32, name="bq4", tag="bq4")
    nc.sync.dma_start(bq4[:], bq_d.rearrange("(j p) -> p j", p=128))
    bq_sb = [bq4[:, j:j + 1] for j in range(4)]

    bv0 = const.tile([1, 512], F32, tag="bv0")
    nc.sync.dma_start(bv0[:], bv_d.unsqueeze(0))
    bv_rep = const.tile([64, 512], F32, tag="bvrep")
    nc.gpsimd.partition_broadcast(bv_rep[:], bv0[:])

    id64r = const.tile([64, 64], F32R, tag="id64r")
    nc.vector.tensor_copy(id64r[:], idx_sb[0:64, 0:64])
    iir = const.tile([128, 64], F32R, tag="iir")
    nc.vector.tensor_copy(iir[:], ii_sb[:])
    z32 = const.tile([128, 8], F32, tag="z32")
    nc.vector.memset(z32[:], 0.0)
    ones32 = const.tile([1, 512], F32, tag="ones32")
    nc.vector.memset(ones32[:], 1.0)

    # persistent per-image state
    xT = [xtp.tile([128, 4, NPIX], BF16, name=f"xT{b}", tag=f"xT{b}")
          for b in range(BPC)]

    # at-tiles: ones row written once, [0:64] rewritten per block
    at_tiles = [atpool.tile([DV + 1, 512], F32R, name=f"at{i}", tag=f"at{i}")
                for i in range(2)]
    for i in range(2):
        nc.scalar.activation(at_tiles[i][64:65, :], ones32[:],
                             mybir.ActivationFunctionType.Copy)

    # ---------------- per-image phases ----------------
    def issue_x_dma(b, g):
        xt = xload.tile([128, 4, 512], BF16, tag="xt")
        nc.sync.dma_start(
            xt[:], x_d[b, g * 512:(g + 1) * 512, :]
            .rearrange("(t p) c -> p t c", p=128))
        return xt

    def load_macro(b, g, xs32, xt=None):
        """One 4-pixel-tile macro: transposes + evac + pixel-sums.
        Rotate transpose psum over pools idle in this phase."""
        if xt is None:
            xt = issue_x_dma(b, g)
        if b == 0:
            rot = [(ps_tp, "tp"), (ps_tp, "tp"), (ps_v, "v"), (ps_v, "v"),
                   (ps_sc, "scd"), (ps_nhv, "nhv")]
        else:
            rot = [(ps_tp, "tp"), (ps_tp, "tp"),
                   (ps_sc, "scd"), (ps_nhv, "nhv")]
        ri = 0
        for t in range(4 * g, 4 * g + 4):
            for half in range(2):
                pool, ptag = rot[(8 * (g % len(rot)) + ri) % len(rot)]
                ri += 1
                ps = pool.tile([128, 2, 129], F32, tag=ptag)
                for j in range(2):
                    jj = half * 2 + j
                    nc.tensor.matmul(ps[:, j, :],
                                     xt[:, t % 4, jj * 128:(jj + 1) * 128],
                                     idx_sb[:], start=True, stop=True)
                dst = xT[b][:, half * 2:half * 2 + 2, t * 128:(t + 1) * 128]
                srcv = ps[:, :, 0:128]
                if (2 * t + half) % 4 != 3:
                    nc.scalar.activation(dst, srcv,
                                         mybir.ActivationFunctionType.Copy)
                else:
                    nc.vector.tensor_copy(dst, srcv)
                sums = ps[:, :, 128:129]
                v = xs32[:, half * 2:half * 2 + 2].unsqueeze(2)
                if t == 0:
                    nc.vector.tensor_copy(v, sums)
                else:
                    nc.vector.tensor_add(v, v, sums)

    def phase_load(b, prefetched=None):
        xs32 = small.tile([128, 4], F32, name=f"xs{b}", tag=f"xs{b}")
        for g in range(NBLK):
            load_macro(b, g, xs32,
                       prefetched[g] if prefetched is not None else None)
        return xs32

    def phase_q(b, xs32):
        """q = (xsum/4096) @ Wq + bq; wqk[c, n] fold (bf16)."""
        xsb = small.tile([128, 4], BF16, tag="xsb")
        nc.vector.tensor_copy(xsb[:], xs32[:])
        qt_sb = []
        for jo in range(4):
            qp = ps_sc.tile([128, 1], F32, tag="scd")
            for j in range(4):
                nc.tensor.matmul(qp[:],
                                 wq_sb[j][:, jo * 128:(jo + 1) * 128],
                                 xsb[:, j:j + 1],
                                 start=(j == 0), stop=(j == 3))
            qt = small.tile([128, 1], F32, tag=f"qt{jo}")
            nc.scalar.activation(qt[:], qp[:],
                                 mybir.ActivationFunctionType.Identity,
                                 bias=bq_sb[jo][:], scale=1.0 / NPIX)
            qt_sb.append(qt)
        qsel = []
        for jo in range(4):
            qs = small.tile([128, 8], BF16, tag=f"qsel{jo}")
            nc.vector.tensor_copy(qs[:], z32[:])
            nc.vector.tensor_copy(qs[0:64, 2 * jo:2 * jo + 1], qt_sb[jo][0:64, :])
            nc.vector.tensor_copy(qs[64:128, 2 * jo + 1:2 * jo + 2],
                                  qt_sb[jo][64:128, :])
            qsel.append(qs)
        wqk = []
        for j in range(4):
            wp = ps_sc.tile([128, 8], F32, tag="scd")
            for jo in range(4):
                nc.tensor.matmul(wp[:], wkT[jo][:, j * 128:(j + 1) * 128],
                                 qsel[jo][:], start=(jo == 0), stop=(jo == 3))
            wq_t = small.tile([128, 8], BF16, tag=f"wqk{j}")
            nc.vector.tensor_copy(wq_t[:], wp[:])
            wqk.append(wq_t)
        return wqk

    def phase_attend(b, wqk):
        """Merged scores + V + wt + nhv/dps, staggered by one e-group."""
        LAG = 4
        nhv = ps_nhv.tile([128, 512], F32, tag="nhv")
        dsum = small.tile([128, 8], F32, name=f"dsum{b}", tag=f"dsum{b}")
        e_groups = [None] * (NTILES // 4)
        scd = None
        for t in range(NTILES + LAG):
            if t < NTILES:
                g, k = t // 4, t % 4
                if k == 0:
                    gpool, gtag = [(ps_sc, "scd"), (ps_tp, "tp")][g % 2]
                    scd = gpool.tile([128, 40], F32, tag=gtag)
                for j in range(4):
                    nc.tensor.matmul(scd[:, k * 8:(k + 1) * 8],
                                     xT[b][:, j, t * 128:(t + 1) * 128],
                                     wqk[j][:], start=(j == 0), stop=(j == 3))
                if k == 3:
                    eg = epool.tile([128, 32], BF16, tag="e")
                    nc.scalar.activation(eg[:], scd[:, 0:32],
                                         mybir.ActivationFunctionType.Exp,
                                         scale=1.0 / np.sqrt(DK))
                    e_groups[g] = eg
                    # per-group closed dps accumulation, drained to SBUF
                    for kk in range(4):
                        nc.tensor.matmul(scd[:, 32:40],
                                         msk_sb[:, 4 * g + kk, :],
                                         eg[:, kk * 8:(kk + 1) * 8],
                                         start=(kk == 0), stop=(kk == 3))
                    if g == 0:
                        nc.vector.tensor_copy(dsum[:], scd[:, 32:40])
                    else:
                        nc.vector.tensor_add(dsum[:], dsum[:], scd[:, 32:40])
            if t >= LAG:
                tv = t - LAG
                eg = e_groups[tv // 4]
                eslice = eg[:, (tv % 4) * 8:(tv % 4) * 8 + 8]
                vp = ps_v.tile([128, 512], F32, tag="v")
                for j in range(4):
                    nc.tensor.matmul(vp[:], xT[b][:, j, tv * 128:(tv + 1) * 128],
                                     wv_sb[j][:], start=(j == 0), stop=(j == 3))
                wt = wpool.tile([128, 512], BF16, tag="w")
                nc.vector.tensor_tensor(
                    wt[:].rearrange("p (n v) -> p n v", n=8),
                    vp[:].rearrange("p (n v) -> p n v", n=8),
                    eslice.unsqueeze(2).broadcast_to([128, 8, 64]),
                    op=mybir.AluOpType.mult)
                nc.tensor.matmul(nhv[:], msk_sb[:, tv, :], wt[:],
                                 start=(tv == 0), stop=(tv == NTILES - 1))
        return nhv, dsum

    def phase_norm(b, nhv, dps):
        """A_h, A_v = numerators/denominators + bv; transpose to [nv, h|w]."""
        dr = small.tile([128, 8], F32, tag="dr")
        nc.vector.reciprocal(dr[:], dps[:])  # dps: SBUF dsum
        ah = att.tile([64, 512], F32R, tag="ah")
        av = att.tile([64, 512], F32R, tag="av")
        nc.vector.tensor_tensor(
            ah[:].rearrange("p (n v) -> p n v", n=8),
            nhv[0:64, :].rearrange("p (n v) -> p n v", n=8),
            dr[0:64, :].unsqueeze(2).broadcast_to([64, 8, 64]),
            op=mybir.AluOpType.mult)
        nc.vector.tensor_add(ah[:], ah[:], bv_rep[:])
        nc.vector.tensor_tensor(
            av[:].rearrange("p (n v) -> p n v", n=8),
            nhv[64:128, :].rearrange("p (n v) -> p n v", n=8),
            dr[64:128, :].unsqueeze(2).broadcast_to([64, 8, 64]),
            op=mybir.AluOpType.mult)
        nc.vector.tensor_add(av[:], av[:], bv_rep[:])

        ahT, avT = [], []
        for j in range(4):
            tp = ps_tp.tile([128, 2, 64], F32, tag="tp")
            nc.tensor.matmul(tp[:, 0, :], ah[:, j * 128:(j + 1) * 128],
                             id64r[:], start=True, stop=True)
            nc.tensor.matmul(tp[:, 1, :], av[:, j * 128:(j + 1) * 128],
                             id64r[:], start=True, stop=True)
            t_sb = att.tile([128, 2, 64], F32, tag=f"ahvT{j}")
            nc.vector.tensor_copy(t_sb[:], tp[:])
            ahT.append(t_sb[:, 0, :])
            avT.append(t_sb[:, 1, :])
        return ahT, avT

    def phase_out_unit(b, g, ahT, avT):
        """One block g: combine outer product + output projection + DMA."""
        atp = ps_big.tile([64, 512], F32, tag="big")
        for j in range(4):
            pt = ppool.tile([128, 512], F32R, tag="p")
            eng = nc.gpsimd if j % 2 == 1 else nc.vector
            eng.tensor_tensor(
                pt[:].rearrange("p (h w) -> p h w", h=8),
                ahT[j][:, g * 8:(g + 1) * 8].unsqueeze(2)
                    .broadcast_to([128, 8, 64]),
                avT[j][:].unsqueeze(1).broadcast_to([128, 8, 64]),
                op=mybir.AluOpType.mult)
            nc.tensor.matmul(atp[:], iir[:], pt[:],
                             start=(j == 0), stop=(j == 3))
        at_sb = at_tiles[g % 2]
        nc.scalar.activation(at_sb[0:64, :], atp[:],
                             mybir.ActivationFunctionType.Copy)
        ot4 = wpool.tile([128, 4, 512], BF16, tag="ow")
        for tt in range(4):
            opool, optag = [(ps_big, "big"), (ps_v, "v")][tt % 2]
            op_ = opool.tile([128, 512], F32, tag=optag)
            nc.tensor.matmul(op_[:], at_sb[:, tt * 128:(tt + 1) * 128],
                             woe_sb[:], start=True, stop=True)
            if tt % 2 == 0:
                nc.scalar.activation(ot4[:, tt, :], op_[:],
                                     mybir.ActivationFunctionType.Copy)
            else:
                nc.vector.tensor_copy(ot4[:, tt, :], op_[:])
        nc.sync.dma_start(
            out_d[b, g * 512:(g + 1) * 512, :]
            .rearrange("(t p) c -> p t c", p=128), ot4[:])

    # ---------------- software-pipelined emission ----------------
    xs0 = phase_load(0, _prefetch0)
    wqk0 = phase_q(0, xs0)
    nhv0, dps0 = phase_attend(0, wqk0)
    ahT0, avT0 = phase_norm(0, nhv0, dps0)
    # interleave image 0 output with image 1 load
    xs1 = small.tile([128, 4], F32, name="xs1", tag="xs1")
    for g in range(NBLK):
        phase_out_unit(0, g, ahT0, avT0)
        load_macro(1, g, xs1)
    wqk1 = phase_q(1, xs1)
    nhv1, dps1 = phase_attend(1, wqk1)
    ahT1, avT1 = phase_norm(1, nhv1, dps1)
    for g in range(NBLK):
        phase_out_unit(1, g, ahT1, avT1)

    ctx.close()


_NC_CACHE = None
PROFILE = False
PROFILE_DIR = None


def kernel(**inputs):
    global _NC_CACHE
    x = np.asarray(inputs["x"], dtype=np.float32)
    Wq = np.asarray(inputs["Wq"], dtype=np.float32)
    bq = np.asarray(inputs["bq"], dtype=np.float32)
    Wk = np.asarray(inputs["Wk"], dtype=np.float32)
    bk = np.asarray(inputs["bk"], dtype=np.float32)
    Wv = np.asarray(inputs["Wv"], dtype=np.float32)
    bv = np.asarray(inputs["bv"], dtype=np.float32)
    Wo = np.asarray(inputs["Wo"], dtype=np.float32)
    bo = np.asarray(inputs["bo"], dtype=np.float32)

    if _NC_CACHE is None:
        _NC_CACHE = _build_kernel()
    nc = _NC_CACHE

    woe = np.concatenate([Wo, bo[None, :]], axis=0)
    idext = np.zeros((128, 129), dtype=BF)
    idext[:, 0:128] = np.eye(128, dtype=BF)
    idext[:, 128] = 1
    ii64 = np.tile(np.eye(64, dtype=BF), (2, 1))
    masks = np.zeros((NTILES, 128, 128), dtype=BF)
    for t in range(NTILES):
        masks[t, 0:64, 2 * t] = 1.0        # Sel_h: h == 2t for first h-row
        masks[t, 64:128, 2 * t + 1] = 1.0  # Sel_h: h == 2t+1 for second
        masks[t, :, 64:128] = np.tile(np.eye(64, dtype=BF), (2, 1))
    masks_t = np.ascontiguousarray(masks.transpose(1, 0, 2))
    xbf = x.astype(BF)
    shared = dict(Wq=Wq.astype(BF), WkT=np.ascontiguousarray(Wk.T).astype(BF),
                  Wv=Wv.astype(BF), Wo_ext=woe,
                  bq=bq, bv=bv, idext=idext, ii64=ii64, masks=masks_t)
    in_maps = []
    for c in range(NCORES):
        m = {"x": xbf[c * BPC:(c + 1) * BPC].reshape(BPC, NPIX, C).copy()}
        m.update(shared)
        in_maps.append(m)

    res = bass_utils.run_bass_kernel_spmd(nc, in_maps, core_ids=list(range(NCORES)),
                                          trace=PROFILE, tmpdir=PROFILE_DIR)
    if PROFILE:
        print("HW exec time:", res.exec_time_ns, "ns")
    outs = [np.asarray(res.results[c]["out"], dtype=np.float32)
            .reshape(BPC, H, W, DO) for c in range(NCORES)]
    return np.concatenate(outs, axis=0)


if __name__ == "__main__":
    rng = np.random.default_rng(0)
    ins = {
        "x": rng.standard_normal((B, H, W, C), dtype=np.float32),
        "Wq": rng.standard_normal((C, 512), dtype=np.float32) * 0.04,
        "bq": np.zeros(512, np.float32),
        "Wk": rng.standard_normal((C, 512), dtype=np.float32) * 0.04,
        "bk": np.zeros(512, np.float32),
        "Wv": rng.standard_normal((C, 512), dtype=np.float32) * 0.04,
        "bv": np.zeros(512, np.float32),
        "Wo": rng.standard_normal((64, 512), dtype=np.float32) * 0.1,
        "bo": np.zeros(512, np.float32),
    }
    out = kernel(**ins)
    print("kernel output", out.shape, out.dtype)
